# revision 1
# baseline (speedup 1.0000x reference)
"""Trainium2 Bass kernel for CombinedRepeatCausalLinear (parallel forward).

Computes out[b,e,t] = sum_s x[b,e,s] * W[s,t] + bias[t] where
  W[s,t] = mask(t>=s) * (w0[s]*d0^(t-s) + w1[t]*d1^(t-s))
for S = 2048, x of shape (8, 1024, 2048) fp32.

Strategy (8 NeuronCores, data-parallel over batch):
  - core c gets x[c] (1024 rows); host pre-transposes to xT (2048, 1024) so
    the contraction dim lands on SBUF partitions with contiguous DMAs.
  - W is rank-2 before causal masking: each (128 s) x (512 t) chunk of W is
    generated ON-CHIP by a K=2 matmul from tiny host-precomputed factor
    vectors (per-chunk exponent offsets keep fp32 in range), then boundary
    chunks are multiplied by one of 4 precomputed 0/1 causal masks on DVE.
  - main matmul runs in float32r (full-rate fp32 mode, 1 cyc/row at N=512):
    outT[t,r] = sum_s W[s,t] * xT[s,r], accumulated over s-tiles in PSUM,
    skipping all-zero below-diagonal blocks (272 of 512 matmuls).
  - bias is fused into the PSUM->SBUF copy on the scalar engine
    (activation Identity with per-partition bias).
  - host transposes each core's outT back and stacks.
"""

import numpy as np

import concourse.bass as bass
import concourse.mybir as mybir
import concourse.tile as tile
from concourse import bacc
from concourse.bass_utils import run_bass_kernel_spmd

F32 = mybir.dt.float32
F32R = mybir.dt.float32r

B = 8
E = 1024
S = 2048
DC = 1.0
N_CORES = 8
R = (B * E) // N_CORES      # rows per core = 1024
ST = S // 128               # 16 s-tiles of 128
TB = S // 512               # 4 t-blocks of 512
RB = R // 512               # 2 r-blocks of 512

# chunk list: (si, tb) with si <= 4*tb+3  (40 chunks)
CHUNKS = [(si, tb) for tb in range(TB) for si in range(min(ST, 4 * tb + 4))]
CHUNK_IDX = {c: i for i, c in enumerate(CHUNKS)}
N_CHUNKS = len(CHUNKS)

_PROGRAM = None  # (nc, ...) cache


def _build_program(repeats=1, no_wgen=False, no_store=False, no_xload=False,
                   po_bufs=4, wc_bufs=30, osb_bufs=4, xsplit=1):
    nc = bacc.Bacc("TRN2", target_bir_lowering=False, debug=False,
                   num_devices=N_CORES)

    xT_d = nc.declare_dram_parameter("xT", [S, R], F32, isOutput=False)
    wstat_d = nc.declare_dram_parameter("wstat", [N_CHUNKS, 2, 128], F32,
                                        isOutput=False)
    wmov_d = nc.declare_dram_parameter("wmov", [N_CHUNKS, 2, 512], F32,
                                       isOutput=False)
    masks_d = nc.declare_dram_parameter("masks", [4, 128, 512], F32,
                                        isOutput=False)
    biasT_d = nc.declare_dram_parameter("biasT", [128, ST], F32,
                                        isOutput=False)
    outT_d = nc.declare_dram_parameter("outT", [S, R], F32, isOutput=True)

    with tile.TileContext(nc) as tc:
        with (
            tc.tile_pool(name="xp", bufs=1) as xp,
            tc.tile_pool(name="cst", bufs=1) as cst,
            tc.tile_pool(name="wg", bufs=6) as wg,
            tc.tile_pool(name="wc", bufs=wc_bufs) as wcp,
            tc.tile_pool(name="osb", bufs=osb_bufs) as osb,
            tc.tile_pool(name="pw", bufs=2, space="PSUM") as pwp,
            tc.tile_pool(name="po", bufs=po_bufs, space="PSUM") as pop,
        ):
            mask_sb = []
            for m in range(4):
                mt = cst.tile([128, 512], F32, tag=f"mask{m}")
                nc.gpsimd.dma_start(mt[:], masks_d[m])
                mask_sb.append(mt)
            bias_sb = cst.tile([128, ST], F32, tag="bias")
            nc.gpsimd.dma_start(bias_sb[:], biasT_d[:])

            for rep in range(repeats):
              # resident x tiles: [128 s, 1024 r] per s-tile
              xs = []
              for si in range(ST):
                t = xp.tile([128, R], F32R, tag=f"x{si}", name=f"x{si}_{rep}")
                if not no_xload:
                    for xs_i in range(xsplit):
                        w0c = (R // xsplit) * xs_i
                        w1c = (R // xsplit) * (xs_i + 1)
                        nc.sync.dma_start(
                            t[:, w0c:w1c],
                            xT_d[128 * si:128 * (si + 1), w0c:w1c]
                            .bitcast(F32R))
                xs.append(t)
              def emit_wgen(tb):
                # generate W chunks (si, tb) for t-block tb
                w_sb = []
                for si in range(min(ST, 4 * tb + 4)):
                    w = wcp.tile([128, 512], F32R, tag="wc", name=f"w{tb}_{si}")
                    if no_wgen:
                        nc.gpsimd.memset(w[:], 0.0)
                    else:
                        ci = CHUNK_IDX[(si, tb)]
                        st = wg.tile([2, 128], F32R, tag="wstat", name="st")
                        nc.gpsimd.dma_start(st[:], wstat_d[ci].bitcast(F32R))
                        mv = wg.tile([2, 512], F32R, tag="wmov", name="mv")
                        nc.gpsimd.dma_start(mv[:], wmov_d[ci].bitcast(F32R))
                        psw = pwp.tile([128, 512], F32, tag="pw", name="psw")
                        nc.tensor.matmul(psw[:], st[:], mv[:], start=True,
                                         stop=True)
                        d2 = 4 * tb - si
                        if d2 <= 0:
                            nc.vector.tensor_mul(w[:], psw[:],
                                                 mask_sb[d2 + 3][:])
                        else:
                            nc.vector.tensor_copy(w[:], psw[:])
                    w_sb.append(w)
                return w_sb

              w_by_tb = {0: emit_wgen(0), 1: emit_wgen(1)}
              for tb in range(TB):
                w_sb = w_by_tb.pop(tb)
                for tjl in range(4):
                    tj = 4 * tb + tjl
                    out_sb = osb.tile([128, R], F32, tag="osb")
                    ps = [pop.tile([128, 512], F32, tag="po", name=f"po{rb}")
                          for rb in range(RB)]
                    for si in range(tj + 1):
                        lhsT = w_sb[si][:, 128 * tjl:128 * (tjl + 1)]
                        for rb in range(RB):
                            nc.tensor.matmul(
                                ps[rb][:], lhsT,
                                xs[si][:, 512 * rb:512 * (rb + 1)],
                                start=(si == 0), stop=(si == tj),
                            )
                    for rb in range(RB):
                        nc.scalar.activation(
                            out_sb[:, 512 * rb:512 * (rb + 1)], ps[rb][:],
                            mybir.ActivationFunctionType.Identity,
                            bias=bias_sb[:, tj:tj + 1],
                        )
                    if not no_store:
                        nc.sync.dma_start(
                            outT_d[128 * tj:128 * (tj + 1), :], out_sb[:])
                if tb + 2 < TB:
                    w_by_tb[tb + 2] = emit_wgen(tb + 2)

    nc.compile()
    return nc


def _host_prep(weight, bias, decay_value):
    w0 = weight[0].astype(np.float64)
    w1 = weight[1].astype(np.float64)
    d0 = float(np.clip(np.float32(decay_value[0, 0]), 0.9, 1.0))
    d1 = float(np.clip(np.float32(decay_value[1, 0]), 0.9, 1.0))
    ii = np.arange(128, dtype=np.float64)
    jj = np.arange(512, dtype=np.float64)

    wstat = np.zeros((N_CHUNKS, 2, 128), dtype=np.float32)
    wmov = np.zeros((N_CHUNKS, 2, 512), dtype=np.float32)
    for ci, (si, tb) in enumerate(CHUNKS):
        d2 = 4 * tb - si
        # W[i,j] = w0[i]*d0^(j-i) + w1[j]*d1^(j-i), j-i = 128*d2 + jj - ii
        wstat[ci, 0] = (w0[128 * si:128 * (si + 1)] * d0 ** (-ii / DC)
                        ).astype(np.float32)
        wstat[ci, 1] = (d1 ** ((128 * d2 - ii) / DC)).astype(np.float32)
        wmov[ci, 0] = (d0 ** ((128 * d2 + jj) / DC)).astype(np.float32)
        wmov[ci, 1] = (w1[512 * tb:512 * (tb + 1)] * d1 ** (jj / DC)
                       ).astype(np.float32)

    masks = np.zeros((4, 128, 512), dtype=np.float32)
    for m in range(4):
        d2 = m - 3
        masks[m] = (128 * d2 + jj[None, :] - ii[:, None] >= 0
                    ).astype(np.float32)

    biasT = np.ascontiguousarray(
        bias.astype(np.float32).reshape(ST, 128).T)
    return wstat, wmov, masks, biasT


def kernel(x, weight, bias, decay_value, index=0, recurrent=0, **_):
    global _PROGRAM
    x = np.asarray(x, dtype=np.float32)
    weight = np.asarray(weight, dtype=np.float32)
    bias = np.asarray(bias, dtype=np.float32)
    decay_value = np.asarray(decay_value, dtype=np.float32)

    if _PROGRAM is None:
        _PROGRAM = _build_program()
    nc = _PROGRAM

    wstat, wmov, masks, biasT = _host_prep(weight, bias, decay_value)

    x2 = x.reshape(B * E, S)
    in_maps = []
    for c in range(N_CORES):
        xT_c = np.ascontiguousarray(x2[R * c:R * (c + 1), :].T)
        in_maps.append({
            "xT": xT_c, "wstat": wstat, "wmov": wmov,
            "masks": masks, "biasT": biasT,
        })

    res = run_bass_kernel_spmd(nc, in_maps, core_ids=list(range(N_CORES)))
    out = np.empty((B * E, S), dtype=np.float32)
    for c in range(N_CORES):
        out[R * c:R * (c + 1), :] = res.results[c]["outT"].T
    return out.reshape(B, E, S)



# revision 3
# speedup vs baseline: 1.8119x; 1.8119x over previous
"""Trainium2 Bass kernel for CombinedRepeatCausalLinear (parallel forward).

Computes out[b,e,t] = sum_s x[b,e,s] * W[s,t] + bias[t] where
  W[s,t] = mask(t>=s) * (w0[s]*d0^(t-s) + w1[t]*d1^(t-s))
for S = 2048, x of shape (8, 1024, 2048) fp32.

W is a causally-masked rank-2 matrix, so x @ W is a 2-state linear
recurrence along t, evaluated as a chunked scan: 17 diagonal chunks of
L=126 columns. Each chunk is ONE [128x128]-stationary matmul per
512-wide r-block:
  stationary rows <- 126 causal-decay x-rows + 2 incoming-carry rows
  psum cols       <- 126 out columns + 2 outgoing-carry (A,B) values
The outgoing carry is copied (DVE) into the next chunk's moving
operand; the two r-blocks form independent chains that interleave on
the PE to hide the serial dependency.

Compute-engine SBUF accesses must start at a 32-aligned partition, so
the carry rows sit at partitions 96..97 and the x/out rows are
permuted to partitions [0..95, 98..127] (the stationary matrix is
host-permuted to match; DMA has no alignment restriction). The last
chunk has 32 rows with carries at partitions 32..33.

Per core (data-parallel over batch rows, 1024 rows/core): 34 matmuls
(vs ~312 for dense-triangular) and fp16 I/O (4MB x in, 4MB out,
0.5MB W) — memory-bound at ~9MB of HBM traffic.
"""

import numpy as np

import concourse.bass as bass
import concourse.mybir as mybir
import concourse.tile as tile
from concourse import bacc
from concourse.bass_utils import run_bass_kernel_spmd

F32 = mybir.dt.float32
F16 = mybir.dt.float16

B = 8
E = 1024
S = 2048
DC = 1.0
N_CORES = 8
R = (B * E) // N_CORES      # rows per core = 1024
L = 126                     # chunk length (+2 carry rows = 128 partitions)
RB = 2                      # r-blocks of 512 (fp32-psum bank width)

CHUNKS = []
_s0 = 0
while _s0 < S:
    CHUNKS.append((_s0, min(L, S - _s0)))
    _s0 += L
NCH = len(CHUNKS)           # 17 (16 x 126 + 1 x 32)

# partition permutation for full chunks: index p -> logical row
# rows 0..95 -> s_rel 0..95; 96,97 -> carries; 98..127 -> s_rel 96..125
PERM = list(range(96)) + [126, 127] + list(range(96, 126))


def _cpos(c):
    """Partition index where chunk c's carry rows live."""
    return 96 if CHUNKS[c][1] == L else 32


_PROGRAM = None


def _build_program():
    nc = bacc.Bacc("TRN2", target_bir_lowering=False, debug=False,
                   num_devices=N_CORES)

    xT_d = nc.declare_dram_parameter("xT", [S, R], F16, isOutput=False)
    waug_d = nc.declare_dram_parameter("waug", [NCH, 128, 128], F16,
                                       isOutput=False)
    biasT_d = nc.declare_dram_parameter("biasT", [128, NCH], F32,
                                        isOutput=False)
    outT_d = nc.declare_dram_parameter("outT", [S, R], F16, isOutput=True)

    with tile.TileContext(nc) as tc:
        with (
            tc.tile_pool(name="cst", bufs=1) as cst,
            tc.tile_pool(name="xp", bufs=1) as xp,
            tc.tile_pool(name="op", bufs=4) as op,
            tc.tile_pool(name="ps", bufs=6, space="PSUM") as psp,
        ):
            wt = []
            for c in range(NCH):
                t = cst.tile([128, 128], F16, tag=f"w{c}", name=f"w{c}")
                nc.sync.dma_start(t[:], waug_d[c])
                wt.append(t)
            bias_sb = cst.tile([128, NCH], F32, tag="bias")
            nc.sync.dma_start(bias_sb[:], biasT_d[:])

            xt = []
            for c, (s0, Lc) in enumerate(CHUNKS):
                t = xp.tile([128, R], F16, tag=f"x{c}", name=f"x{c}")
                if Lc == L:
                    nc.sync.dma_start(t[0:96, :], xT_d[s0:s0 + 96, :])
                    nc.sync.dma_start(t[98:128, :], xT_d[s0 + 96:s0 + Lc, :])
                else:
                    nc.sync.dma_start(t[0:Lc, :], xT_d[s0:s0 + Lc, :])
                xt.append(t)
            cp0 = _cpos(0)
            nc.vector.memset(xt[0][cp0:cp0 + 2, :], 0.0)

            for c, (s0, Lc) in enumerate(CHUNKS):
                last = (c + 1 == NCH)
                K = Lc + 2
                M = Lc if last else Lc + 2
                ot = op.tile([128, R], F16, tag="ot", name=f"ot{c}")
                for rb in range(RB):
                    rbs = slice(512 * rb, 512 * (rb + 1))
                    ps = psp.tile([128, 512], F32, tag="ps",
                                  name=f"ps{c}_{rb}")
                    nc.tensor.matmul(ps[0:M, :], wt[c][0:K, 0:M],
                                     xt[c][0:K, rbs], start=True, stop=True)
                    if not last:
                        cp = _cpos(c + 1)
                        nc.vector.tensor_copy(xt[c + 1][cp:cp + 2, rbs],
                                              ps[96:98, :])
                    if Lc == L:
                        nc.scalar.activation(
                            ot[0:96, rbs], ps[0:96, :],
                            mybir.ActivationFunctionType.Identity,
                            bias=bias_sb[0:96, c:c + 1])
                        nc.scalar.activation(
                            ot[96:128, rbs], ps[96:128, :],
                            mybir.ActivationFunctionType.Identity,
                            bias=bias_sb[96:128, c:c + 1])
                    else:
                        nc.scalar.activation(
                            ot[0:Lc, rbs], ps[0:Lc, :],
                            mybir.ActivationFunctionType.Identity,
                            bias=bias_sb[0:Lc, c:c + 1])
                if Lc == L:
                    nc.scalar.dma_start(outT_d[s0:s0 + 96, :], ot[0:96, :])
                    nc.scalar.dma_start(outT_d[s0 + 96:s0 + Lc, :],
                                        ot[98:128, :])
                else:
                    nc.scalar.dma_start(outT_d[s0:s0 + Lc, :], ot[0:Lc, :])

    nc.compile()
    return nc


def _host_prep(weight, bias, decay_value):
    w0 = weight[0].astype(np.float64)
    w1 = weight[1].astype(np.float64)
    d0 = float(np.clip(np.float32(decay_value[0, 0]), 0.9, 1.0)) ** (1.0 / DC)
    d1 = float(np.clip(np.float32(decay_value[1, 0]), 0.9, 1.0)) ** (1.0 / DC)

    waug = np.zeros((NCH, 128, 128), dtype=np.float64)
    biasT = np.zeros((128, NCH), dtype=np.float32)
    for c, (s0, Lc) in enumerate(CHUNKS):
        # logical layout: rows 0..Lc-1 = x rows, Lc..Lc+1 = carry-in;
        # cols 0..Lc-1 = out cols, Lc..Lc+1 = carry-out (dropped on last)
        w = np.zeros((128, 128))
        ii = np.arange(Lc)
        jj = np.arange(Lc)
        msk = jj[None, :] >= ii[:, None]
        pw = np.where(msk, jj[None, :] - ii[:, None], 0)
        w[:Lc, :Lc] = np.where(
            msk,
            w0[s0 + ii][:, None] * d0 ** pw + w1[s0 + jj][None, :] * d1 ** pw,
            0.0)
        w[Lc, :Lc] = d0 ** (jj + 1)
        w[Lc + 1, :Lc] = w1[s0 + jj] * d1 ** (jj + 1)
        last = (c + 1 == NCH)
        if not last:
            w[:Lc, Lc] = w0[s0 + ii] * d0 ** (Lc - 1 - ii)
            w[:Lc, Lc + 1] = d1 ** (Lc - 1 - ii)
            w[Lc, Lc] = d0 ** Lc
            w[Lc + 1, Lc + 1] = d1 ** Lc
        bcol = np.zeros(128, dtype=np.float32)
        bcol[:Lc] = bias[s0:s0 + Lc]
        if Lc == L:
            # permute rows/cols so carries land on partitions 96..97
            w = w[np.ix_(PERM, PERM)]
            bcol = bcol[PERM]
        waug[c] = w
        biasT[:, c] = bcol
    return waug.astype(np.float16), biasT


def prep_in_maps(x, weight, bias, decay_value):
    waug, biasT = _host_prep(weight, bias, decay_value)
    x2 = np.asarray(x, dtype=np.float32).reshape(B * E, S)
    in_maps = []
    for c in range(N_CORES):
        xT_c = np.ascontiguousarray(
            x2[R * c:R * (c + 1), :].astype(np.float16).T)
        in_maps.append({"xT": xT_c, "waug": waug, "biasT": biasT})
    return in_maps


def kernel(x, weight, bias, decay_value, index=0, recurrent=0, **_):
    global _PROGRAM
    x = np.asarray(x, dtype=np.float32)
    weight = np.asarray(weight, dtype=np.float32)
    bias = np.asarray(bias, dtype=np.float32)
    decay_value = np.asarray(decay_value, dtype=np.float32)

    if _PROGRAM is None:
        _PROGRAM = _build_program()

    in_maps = prep_in_maps(x, weight, bias, decay_value)
    res = run_bass_kernel_spmd(_PROGRAM, in_maps,
                               core_ids=list(range(N_CORES)))
    out = np.empty((B * E, S), dtype=np.float32)
    for c in range(N_CORES):
        oT = res.results[c]["outT"]
        out[R * c:R * (c + 1), :] = oT.T.astype(np.float32)
    return out.reshape(B, E, S)


# revision 5
# speedup vs baseline: 3.4985x; 1.9308x over previous
"""Trainium2 Bass kernel for CombinedRepeatCausalLinear (parallel forward).

Computes out[b,e,t] = sum_s x[b,e,s] * W[s,t] + bias[t] where
  W[s,t] = mask(t>=s) * (w0[s]*d0^(t-s) + w1[t]*d1^(t-s))
for S = 2048, x of shape (8, 1024, 2048) fp32.

W is a causally-masked rank-2 matrix, so x @ W is a 2-state linear
recurrence along t, evaluated as a chunked scan: 17 diagonal chunks of
L=126 columns. Each chunk is ONE [128x128]-stationary matmul per
512-wide r-block:
  stationary rows <- 126 causal-decay x-rows + 2 incoming-carry rows
  psum cols       <- 126 out columns + 2 outgoing-carry (A,B) values
The outgoing carry is copied (DVE) into the next chunk's moving
operand; the two r-blocks form independent chains that interleave on
the PE to hide the serial dependency.

Layout/perf notes:
- Compute-engine SBUF accesses must start at a 32-aligned partition,
  so carry rows sit at partitions 96..97 and x/out rows occupy
  [0..95, 98..127] (stationary host-permuted to match). Last chunk
  (32 rows) keeps carries at 32..33.
- fp16 end-to-end I/O: 4MB x in + 4MB out + 0.5MB W per core.
- x and out are PACKED 4 chunks per DMA instruction ([128, 4096] fp16
  tiles = 8KB contiguous per partition) because per-queue DMA
  throughput is packet-size-bound; W is one packed [128, 2176] DMA.
- Input/output transfers are split across both HWDGE queues
  (sync + scalar).
- ~10 dummy matmuls run during the initial DMA wait to warm the PE
  clock (HAM un-throttle) before the serial chain starts.
"""

import numpy as np

import concourse.bass as bass
import concourse.mybir as mybir
import concourse.tile as tile
from concourse import bacc
from concourse.bass_utils import run_bass_kernel_spmd

F32 = mybir.dt.float32
F16 = mybir.dt.float16

B = 8
E = 1024
S = 2048
DC = 1.0
N_CORES = 8
R = (B * E) // N_CORES      # rows per core = 1024
L = 126                     # chunk length (+2 carry rows = 128 partitions)
RB = 2                      # r-blocks of 512 (fp32-psum bank width)
PACKW = 4                   # chunks per packed DMA
NPACK = 4                   # packs of full chunks (16 full chunks)
NFULL = NPACK * PACKW       # 16
SLAST = NFULL * L           # 2016
LLAST = S - SLAST           # 32
NCH = NFULL + 1             # 17
NDUMMY = 10                 # PE warm-up matmuls

# partition p -> s_rel within a full chunk (96..97 are carry slots)
SREL = list(range(96)) + [None, None] + list(range(96, L))
# logical row/col permutation applied to the [128,128] stationary block
PERM = list(range(96)) + [126, 127] + list(range(96, 126))

_PROGRAM = None


def _build_program():
    nc = bacc.Bacc("TRN2", target_bir_lowering=False, debug=False,
                   num_devices=N_CORES)

    xpk_d = nc.declare_dram_parameter("xpk", [128, NPACK, PACKW * R], F16,
                                      isOutput=False)
    xlast_d = nc.declare_dram_parameter("xlast", [LLAST, R], F16,
                                        isOutput=False)
    wpk_d = nc.declare_dram_parameter("wpk", [128, NCH * 128], F16,
                                      isOutput=False)
    biasT_d = nc.declare_dram_parameter("biasT", [128, NCH], F32,
                                        isOutput=False)
    opk_d = nc.declare_dram_parameter("opk", [128, NPACK, PACKW * R], F16,
                                      isOutput=True)
    olast_d = nc.declare_dram_parameter("olast", [LLAST, R], F16,
                                        isOutput=True)

    with tile.TileContext(nc) as tc:
        with (
            tc.tile_pool(name="cst", bufs=1) as cst,
            tc.tile_pool(name="xp", bufs=1) as xp,
            tc.tile_pool(name="op", bufs=1) as op,
            tc.tile_pool(name="dum", bufs=1) as dum,
            tc.tile_pool(name="ps", bufs=6, space="PSUM") as psp,
            tc.tile_pool(name="pd", bufs=2, space="PSUM") as pdp,
        ):
            # ---- PE warm-up: dummy matmuls on memset tiles ----
            wdum = dum.tile([128, 128], F16, tag="wdum")
            xdum = dum.tile([128, 512], F16, tag="xdum")
            nc.vector.memset(wdum[:], 0.0)
            nc.vector.memset(xdum[:], 0.0)
            for i in range(NDUMMY):
                pd = pdp.tile([128, 512], F32, tag="pd", name=f"pd{i}")
                nc.tensor.matmul(pd[:], wdum[:], xdum[:],
                                 start=True, stop=True)

            # ---- constants: packed W + bias (scalar queue, first) ----
            wpk = cst.tile([128, NCH * 128], F16, tag="wpk")
            nc.scalar.dma_start(wpk[:], wpk_d[:])
            bias_sb = cst.tile([128, NCH], F32, tag="bias")
            nc.scalar.dma_start(bias_sb[:], biasT_d[:])

            # ---- x input: 4 packs split across the two HWDGE queues ----
            xt = []
            for q in range(NPACK):
                t = xp.tile([128, PACKW * R], F16, tag=f"xq{q}",
                            name=f"xq{q}")
                eng = nc.sync if q < 2 else nc.scalar
                eng.dma_start(t[:], xpk_d[:, q, :])
                xt.append(t)
            xlast = xp.tile([128, R], F16, tag="xlast")
            nc.sync.dma_start(xlast[0:LLAST, :], xlast_d[:])

            # chunk 0's carry-in rows arrive zeroed from the host pack
            ot = [op.tile([128, PACKW * R], F16, tag=f"oq{q}", name=f"oq{q}")
                  for q in range(NPACK)]
            olast = op.tile([128, R], F16, tag="olast")

            for c in range(NCH):
                last = (c == NCH - 1)
                q, k = c // PACKW, c % PACKW
                if last:
                    K, M = LLAST + 2, LLAST
                    mov, dst = xlast, olast
                    koff = 0
                else:
                    K, M = 128, 128
                    mov, dst = xt[q], ot[q]
                    koff = k * R
                for rb in range(RB):
                    fs = slice(koff + 512 * rb, koff + 512 * (rb + 1))
                    ps = psp.tile([128, 512], F32, tag="ps",
                                  name=f"ps{c}_{rb}")
                    nc.tensor.matmul(ps[0:M, :],
                                     wpk[0:K, 128 * c:128 * c + M],
                                     mov[0:K, fs], start=True, stop=True)
                    if c + 1 < NCH:
                        nq, nk = (c + 1) // PACKW, (c + 1) % PACKW
                        if c + 1 == NCH - 1:
                            ndst = xlast
                            cp, nfs = LLAST, slice(512 * rb, 512 * (rb + 1))
                        else:
                            ndst = xt[nq]
                            cp = 96
                            nfs = slice(nk * R + 512 * rb,
                                        nk * R + 512 * (rb + 1))
                        nc.vector.tensor_copy(ndst[cp:cp + 2, nfs],
                                              ps[96:98, :])
                    nc.scalar.activation(
                        dst[0:M, fs], ps[0:M, :],
                        mybir.ActivationFunctionType.Identity,
                        bias=bias_sb[0:M, c:c + 1])
                if last:
                    nc.sync.dma_start(olast_d[:], olast[0:LLAST, :])
                elif k == PACKW - 1:
                    eng = nc.sync if q < 2 else nc.scalar
                    eng.dma_start(opk_d[:, q, :], ot[q][:])

    nc.compile()
    return nc


def _host_prep(weight, bias, decay_value):
    w0 = weight[0].astype(np.float64)
    w1 = weight[1].astype(np.float64)
    d0 = float(np.clip(np.float32(decay_value[0, 0]), 0.9, 1.0)) ** (1.0 / DC)
    d1 = float(np.clip(np.float32(decay_value[1, 0]), 0.9, 1.0)) ** (1.0 / DC)

    wpk = np.zeros((128, NCH * 128), dtype=np.float64)
    biasT = np.zeros((128, NCH), dtype=np.float32)
    for c in range(NCH):
        s0 = c * L
        Lc = L if c < NFULL else LLAST
        w = np.zeros((128, 128))
        ii = np.arange(Lc)
        jj = np.arange(Lc)
        msk = jj[None, :] >= ii[:, None]
        pw = np.where(msk, jj[None, :] - ii[:, None], 0)
        w[:Lc, :Lc] = np.where(
            msk,
            w0[s0 + ii][:, None] * d0 ** pw + w1[s0 + jj][None, :] * d1 ** pw,
            0.0)
        w[Lc, :Lc] = d0 ** (jj + 1)
        w[Lc + 1, :Lc] = w1[s0 + jj] * d1 ** (jj + 1)
        if c < NFULL:
            w[:Lc, Lc] = w0[s0 + ii] * d0 ** (Lc - 1 - ii)
            w[:Lc, Lc + 1] = d1 ** (Lc - 1 - ii)
            w[Lc, Lc] = d0 ** Lc
            w[Lc + 1, Lc + 1] = d1 ** Lc
        bcol = np.zeros(128, dtype=np.float32)
        bcol[:Lc] = bias[s0:s0 + Lc]
        if c < NFULL:
            w = w[np.ix_(PERM, PERM)]
            bcol = bcol[PERM]
        wpk[:, 128 * c:128 * (c + 1)] = w
        biasT[:, c] = bcol
    return wpk.astype(np.float16), biasT


# gather indices: IDX[p, c] = global s row for partition p of full chunk c
_IDX = np.zeros((128, NFULL), dtype=np.int64)
_VALID = np.ones(128, dtype=bool)
for _p in range(128):
    if SREL[_p] is None:
        _VALID[_p] = False
        continue
    for _c in range(NFULL):
        _IDX[_p, _c] = _c * L + SREL[_p]


def prep_in_maps(x, weight, bias, decay_value):
    wpk, biasT = _host_prep(weight, bias, decay_value)
    x2 = np.asarray(x, dtype=np.float32).reshape(B * E, S)
    in_maps = []
    for core in range(N_CORES):
        xc = x2[R * core:R * (core + 1), :].astype(np.float16)
        xT = np.ascontiguousarray(xc.T)              # [S, R]
        xpk = xT[_IDX.T.reshape(-1), :].reshape(NFULL, 128, R)
        xpk = np.ascontiguousarray(xpk.transpose(1, 0, 2)).reshape(
            128, NPACK, PACKW * R)
        xpk[96:98, :, :] = 0
        xlast = np.ascontiguousarray(xT[SLAST:, :])
        in_maps.append({"xpk": xpk, "xlast": xlast, "wpk": wpk,
                        "biasT": biasT})
    return in_maps


def unpack_out(res_c):
    """Reassemble one core's [R, S] fp32 output from packed results."""
    opk = res_c["opk"].reshape(128, NFULL, R)
    outT = np.empty((S, R), dtype=np.float16)
    outT[_IDX[_VALID].reshape(-1), :] = opk[_VALID].reshape(-1, R)
    outT[SLAST:, :] = res_c["olast"]
    return outT.T.astype(np.float32)


def kernel(x, weight, bias, decay_value, index=0, recurrent=0, **_):
    global _PROGRAM
    x = np.asarray(x, dtype=np.float32)
    weight = np.asarray(weight, dtype=np.float32)
    bias = np.asarray(bias, dtype=np.float32)
    decay_value = np.asarray(decay_value, dtype=np.float32)

    if _PROGRAM is None:
        _PROGRAM = _build_program()

    in_maps = prep_in_maps(x, weight, bias, decay_value)
    res = run_bass_kernel_spmd(_PROGRAM, in_maps,
                               core_ids=list(range(N_CORES)))
    out = np.empty((B * E, S), dtype=np.float32)
    for c in range(N_CORES):
        out[R * c:R * (c + 1), :] = unpack_out(res.results[c])
    return out.reshape(B, E, S)
